# revision 23
# baseline (speedup 1.0000x reference)
"""EulerCE attention Trainium2 kernel (v2: fp8 DoubleRow + bf16 + merged exps).

Sharding: data-parallel over batch (2) x head-parallel over 4 head-groups
(16 heads / 4 per group) = 8 cores. Core c: batch c//4, heads 4*(c%4)..+4.

Numerics (validated against a float64 oracle by host-side emulation):
  - Q/K projection in fp8-e4m3 DoubleRow (x*16, W*128 host-scaled), except
    positions 0:128 which get a bf16 "patch" projection (few-key softmax rows
    amplify q/k error; bf16 there drops max rel err from 2.3e-2 to 4.7e-3).
  - RoPE rotation on DVE+Pool in f32 out of PSUM; rotated q/k written
    directly as fp8 planes [128, 2(even/odd), n] for DoubleRow scores —
    scale 8/(16*128) folded into the host cos/sin tables.
  - Scores s^T[k,q] via fp8 DoubleRow (contraction 64 = 32 partitions x 2
    planes per head); decay bias folded into the exp bias column; exp scale
    1/512 = 1/(8*8*sqrt(64)).
  - One exp per (head, key-tile) covering the whole valid 1024-col q range
    of a strip-pair (contiguous over the 2 PSUM banks) -> bf16 P.
  - V path fully bf16 (max-normalized error is dominated by early-q rows
    where attention passes V through unaveraged; fp8 V fails the gate).
  - PV with the 64-ones-column denominator trick; reciprocal_approx_fast +
    tensor_mul -> bf16 attn^T; O-projection bf16; output DMA'd from PSUM.
"""

import sys

sys.path.insert(0, "/opt/trn_rl_repo")

import math

import ml_dtypes
import numpy as np

import concourse.bass as bass
from concourse import bacc
import concourse.mybir as mybir
import concourse.tile as tile
from concourse.bass_utils import run_bass_kernel_spmd

F32 = mybir.dt.float32
BF16 = mybir.dt.bfloat16
F8 = mybir.dt.float8e4
EXP = mybir.ActivationFunctionType.Exp
DR = mybir.MatmulPerfMode.DoubleRow

D_MODEL = 1024
N_HEADS = 16
D_HEAD = 64
BATCH = 2
SEQ = 2048
H_LOC = 4          # heads per core
CH = 512           # proj n-chunk size
KT = 128           # key tile
NT = SEQ // KT     # 16 key tiles
SP_W = 1024        # strip-pair q width
PATCH = 128        # bf16-patched positions

SX = 16.0          # fp8 scale on x
SW = 128.0         # fp8 scale on Wq/Wk
SQ = 8.0           # fp8 scale on rotated q/k
EXP_SCALE = 1.0 / (SQ * SQ * 8.0)


DEBUG = False


def build_program(reps=1):
    nc = bacc.Bacc()
    xf8 = nc.dram_tensor("xf8", [D_MODEL, SEQ], F8, kind="ExternalInput")
    xb16 = nc.dram_tensor("xb16", [D_MODEL, SEQ], BF16, kind="ExternalInput")
    wqk8 = nc.dram_tensor("wqk8", [D_MODEL, 512], F8, kind="ExternalInput")
    wqk16 = nc.dram_tensor("wqk16", [D_MODEL, 512], BF16, kind="ExternalInput")
    wv = nc.dram_tensor("wv", [D_MODEL, 256], BF16, kind="ExternalInput")
    wo = nc.dram_tensor("wo", [256, D_MODEL], BF16, kind="ExternalInput")
    cost = nc.dram_tensor("cost", [128, SEQ], F32, kind="ExternalInput")
    sint = nc.dram_tensor("sint", [128, SEQ], F32, kind="ExternalInput")
    biast = nc.dram_tensor("biast", [128, H_LOC * NT], F32, kind="ExternalInput")
    idm = nc.dram_tensor("idm", [64, 2, 128], F8, kind="ExternalInput")
    trim = nc.dram_tensor("trim", [64, 2, 128], F8, kind="ExternalInput")
    out = nc.dram_tensor("out", [SEQ, D_MODEL], F32, kind="ExternalOutput")
    dbg_q = nc.dram_tensor("dbg_q", [128, 2, SEQ], F8, kind="ExternalOutput") if DEBUG else None
    dbg_k = nc.dram_tensor("dbg_k", [128, 2, SEQ], F8, kind="ExternalOutput") if DEBUG else None
    dbg_v = nc.dram_tensor("dbg_v", [128, NT, H_LOC, 128], BF16, kind="ExternalOutput") if DEBUG else None
    dbg_at = nc.dram_tensor("dbg_at", [128, 2, 2, SP_W], BF16, kind="ExternalOutput") if DEBUG else None
    dbg_pt = nc.dram_tensor("dbg_pt", [128, SP_W], BF16, kind="ExternalOutput") if DEBUG else None

    with tile.TileContext(nc) as tc:
        with (
            tc.tile_pool(name="consts", bufs=1) as consts,
            tc.tile_pool(name="persist", bufs=1) as persist,
            tc.tile_pool(name="xch8", bufs=2) as xchp8,
            tc.tile_pool(name="xch16", bufs=2) as xchp16,
            tc.tile_pool(name="rot", bufs=2) as rotp,
            tc.tile_pool(name="grot", bufs=2) as grotp,
            tc.tile_pool(name="ptp", bufs=5) as ptp,
            tc.tile_pool(name="attnp", bufs=2) as attnp,
            tc.tile_pool(name="recp", bufs=2) as recp,
            tc.tile_pool(name="obp", bufs=3) as obp,
            tc.tile_pool(name="sps", bufs=2, space="PSUM") as sps,
            tc.tile_pool(name="avps", bufs=2, space="PSUM") as avps,
        ):
            # ---- constants in ----
            wqk8_sb = consts.tile([128, 8, 512], F8, tag="wqk8")
            nc.sync.dma_start(out=wqk8_sb, in_=wqk8.rearrange("(k p) m -> p k m", p=128))
            wqk16_sb = consts.tile([128, 8, 512], BF16, tag="wqk16")
            nc.scalar.dma_start(out=wqk16_sb, in_=wqk16.rearrange("(k p) m -> p k m", p=128))
            wv_sb = consts.tile([128, 8, 256], BF16, tag="wv")
            nc.scalar.dma_start(out=wv_sb, in_=wv.rearrange("(k p) m -> p k m", p=128))
            wo_sb = consts.tile([128, 2, D_MODEL], BF16, tag="wo")
            nc.scalar.dma_start(out=wo_sb, in_=wo.rearrange("(k p) m -> p k m", p=128))
            cos_sb = consts.tile([128, SEQ], F32, tag="cos")
            nc.sync.dma_start(out=cos_sb, in_=cost[:, :])
            sin_sb = consts.tile([128, SEQ], F32, tag="sin")
            nc.sync.dma_start(out=sin_sb, in_=sint[:, :])
            bias_sb = consts.tile([128, H_LOC * NT], F32, tag="bias")
            nc.scalar.dma_start(out=bias_sb, in_=biast[:, :])
            idm_sb = consts.tile([64, 2, 128], F8, tag="idm")
            nc.scalar.dma_start(out=idm_sb, in_=idm[:, :, :])
            trim_sb = consts.tile([64, 2, 128], F8, tag="trim")
            nc.scalar.dma_start(out=trim_sb, in_=trim[:, :, :])

            # V in [key, dh] layout; cols 64:128 ones (denominator trick)
            v_sb = persist.tile([128, NT, H_LOC, 128], BF16, tag="vsb")
            nc.vector.memset(v_sb[:, :, :, 64:128], 1.0)

            # rotated q/k, fp8 planes: [128p(=4 heads x 32), 2(even/odd), n]
            qf8 = persist.tile([128, 2, SEQ], F8, tag="qf8")
            kf8 = persist.tile([128, 2, SEQ], F8, tag="kf8")
            # bf16 patch of rotated q/k for positions 0:PATCH
            qp16 = persist.tile([128, 2, PATCH], BF16, tag="qp16")
            kp16 = persist.tile([128, 2, PATCH], BF16, tag="kp16")

            at_tiles = {}

            def rotate(pe, po, dst, c0):
                # dst fp8 planes; scale folded into cos/sin tables. DVE does
                # the PSUM-reading muls (bf16 intermediates), Pool (no PSUM
                # access) does the SBUF-only combine into fp8.
                cs = cos_sb[:, c0:c0 + CH]
                sn = sin_sb[:, c0:c0 + CH]
                t1 = rotp.tile([128, CH], BF16, tag="t1")
                t2 = rotp.tile([128, CH], BF16, tag="t2")
                t3 = grotp.tile([128, CH], BF16, tag="t3")
                t4 = grotp.tile([128, CH], BF16, tag="t4")
                nc.vector.tensor_mul(t1[:, :], pe[:, :], cs)
                nc.vector.tensor_mul(t2[:, :], po[:, :], sn)
                nc.gpsimd.tensor_sub(dst[:, 0, c0:c0 + CH], t1[:, :], t2[:, :])
                nc.vector.tensor_mul(t3[:, :], pe[:, :], sn)
                nc.vector.tensor_mul(t4[:, :], po[:, :], cs)
                nc.gpsimd.tensor_add(dst[:, 1, c0:c0 + CH], t3[:, :], t4[:, :])

            def patch_block(xc16):
                # bf16 proj + rotate for positions 0:PATCH (chunk 0 only);
                # all 4 M-blocks packed into one PSUM bank (sequential chains)
                # HW PSUM zero-regions are bank-wide: arm the bank once (the
                # very first matmul), later chains rely on zero-on-first-touch
                # of still-pending bytes, so their start stays False.
                pfull = avps.tile([128, 2, CH], F32, tag="avp", name="patch")
                pq = []
                for m in range(4):
                    p = pfull[:, 0, m * PATCH:(m + 1) * PATCH]
                    for j in range(8):
                        nc.tensor.matmul(
                            p,
                            wqk16_sb[:, j, m * 128:(m + 1) * 128],
                            xc16[:, j, 0:PATCH],
                            start=(m == 0 and j == 0), stop=(j == 7),
                            skip_group_check=True,
                        )
                    pq.append(p)
                cs = cos_sb[:, 0:PATCH]
                sn = sin_sb[:, 0:PATCH]
                for pe, po, dst in ((pq[0], pq[1], qp16), (pq[2], pq[3], kp16)):
                    t1 = rotp.tile([128, PATCH], F32, tag="t1")
                    t2 = rotp.tile([128, PATCH], F32, tag="t2")
                    nc.vector.tensor_mul(t1[:, :], pe, cs)
                    nc.vector.tensor_mul(t2[:, :], po, sn)
                    nc.vector.tensor_sub(dst[:, 0, :], t1[:, :], t2[:, :])
                    t3 = grotp.tile([128, PATCH], F32, tag="t3p")
                    t4 = grotp.tile([128, PATCH], F32, tag="t4p")
                    nc.vector.tensor_mul(t3[:, :], pe, sn)
                    nc.vector.tensor_mul(t4[:, :], po, cs)
                    nc.vector.tensor_add(dst[:, 1, :], t3[:, :], t4[:, :])

            def proj_chunk(c):
                c0 = c * CH
                xc8 = xchp8.tile([128, 8, CH], F8, tag="xc8")
                nc.sync.dma_start(out=xc8, in_=xf8[:, c0:c0 + CH].rearrange("(k p) m -> p k m", p=128))
                xc16 = xchp16.tile([128, 8, CH], BF16, tag="xc16")
                nc.scalar.dma_start(out=xc16, in_=xb16[:, c0:c0 + CH].rearrange("(k p) m -> p k m", p=128))
                for mp, dst in ((0, qf8), (1, kf8)):  # QE/QO then KE/KO, fp8 DR
                    pqk = sps.tile([128, 2, CH], F32, tag="sp", name=f"qk{mp}")
                    for half in range(2):
                        m = 2 * mp + half
                        for j in range(4):
                            nc.tensor.matmul(
                                pqk[:, half, :],
                                wqk8_sb[:, 2 * j:2 * j + 2, m * 128:(m + 1) * 128],
                                xc8[:, 2 * j:2 * j + 2, :],
                                start=(j == 0), stop=(j == 3),
                                perf_mode=DR,
                            )
                    rotate(pqk[:, 0, :], pqk[:, 1, :], dst, c0)
                if c == 0:
                    patch_block(xc16)
                for kt in range(4):  # V projection, bf16
                    t = 4 * c + kt
                    vpf = avps.tile([128, 2, CH], F32, tag="avp", name="vp")
                    vp = vpf[:, 0, 0:256]
                    for j in range(8):
                        nc.tensor.matmul(
                            vp,
                            xc16[:, j, kt * 128:(kt + 1) * 128],
                            wv_sb[:, j, :],
                            start=(j == 0), stop=(j == 7),
                        )
                    nc.vector.tensor_copy(
                        out=v_sb[:, t, :, 0:64],
                        in_=vp.rearrange("p (h d) -> p h d", h=4),
                    )

            def strip_pair(S):
                q0 = S * SP_W
                T = 8 * S + 8  # key tiles in range
                for pair in range(2):
                    at = attnp.tile([128, SP_W], BF16, tag=f"at{pair}",
                                    name=f"at_{S}_{pair}")
                    at_tiles[(S, pair)] = at
                    avs = [
                        avps.tile([128, 2, CH], F32, tag="avp",
                                  name=f"av_{S}_{pair}_{hl}")
                        for hl in range(2)
                    ]
                    def emit_pv(fhl, ft, fpt):
                        fh = 2 * pair + fhl
                        for half in range(2):
                            hq0 = q0 + half * CH
                            qoff = max(0, KT * ft - hq0)
                            if qoff >= CH:
                                continue
                            t_hi = 8 * S + 4 * (half + 1)
                            nc.tensor.matmul(
                                avs[fhl][:, half, qoff:CH],
                                v_sb[:, ft, fh, :],
                                fpt[:, half * CH + qoff:(half + 1) * CH],
                                start=(ft == 0), stop=(ft == t_hi - 1),
                            )

                    pend = []  # deferred PV work: (hl, t, pt)
                    for t in range(T):
                        for hl in range(2):
                            h = 2 * pair + hl
                            r0 = 32 * h
                            sp = sps.tile([128, 2, CH], F32, tag="sp",
                                          name=f"sp{hl}")
                            o = max(0, KT * t - q0)  # first valid col
                            for half in range(2):
                                hq0 = q0 + half * CH
                                qoff = max(0, KT * t - hq0)
                                if qoff >= CH:
                                    continue
                                if S == 0 and t == 0 and half == 0:
                                    # bf16 patch for q 0:128 then fp8 rest
                                    nc.tensor.matmul(
                                        sp[:, 0, 0:PATCH],
                                        kp16[r0:r0 + 32, 0, :],
                                        qp16[r0:r0 + 32, 0, :],
                                        start=True, stop=False,
                                        tile_position=(r0, 0),
                                    )
                                    nc.tensor.matmul(
                                        sp[:, 0, 0:PATCH],
                                        kp16[r0:r0 + 32, 1, :],
                                        qp16[r0:r0 + 32, 1, :],
                                        start=False, stop=True,
                                        tile_position=(r0, 0),
                                    )
                                    # same bank as the patch chain above:
                                    # start=False, bytes are still pending
                                    # from its bank-wide arm (zero-on-touch)
                                    nc.tensor.matmul(
                                        sp[:, 0, PATCH:CH],
                                        kf8[r0:r0 + 32, :, 0:KT],
                                        qf8[r0:r0 + 32, :, PATCH:CH],
                                        start=False, stop=True,
                                        perf_mode=DR,
                                        tile_position=(r0, 0),
                                        skip_group_check=True,
                                    )
                                    nc.tensor.matmul(
                                        sp[:, 0, 0:KT],
                                        idm_sb[:, :, :], trim_sb[:, :, :],
                                        start=False, stop=True,
                                        perf_mode=DR,
                                        skip_group_check=True,
                                    )
                                    continue
                                nc.tensor.matmul(
                                    sp[:, half, qoff:CH],
                                    kf8[r0:r0 + 32, :, t * KT:(t + 1) * KT],
                                    qf8[r0:r0 + 32, :, hq0 + qoff:hq0 + CH],
                                    start=True, stop=True,
                                    perf_mode=DR,
                                    tile_position=(r0, 0),
                                )
                                if KT * t >= q0 and (o // CH) == half:
                                    # causal mask folded into the score PSUM:
                                    # identity(+240) x lower-tri(-240) adds
                                    # -57600 on masked (q < k) elements; exp
                                    # then yields exactly 0 there.
                                    om = o % CH
                                    nc.tensor.matmul(
                                        sp[:, half, om:om + KT],
                                        idm_sb[:, :, :], trim_sb[:, :, :],
                                        start=False, stop=True,
                                        perf_mode=DR,
                                        skip_group_check=True,
                                    )
                            pt = ptp.tile([128, SP_W], BF16, tag="pt",
                                          name=f"pt{hl}")
                            col = h * NT + t
                            spf = sp[:, :, :].rearrange("p a b -> p (a b)")
                            nc.scalar.activation(
                                out=pt[:, o:SP_W], in_=spf[:, o:SP_W],
                                func=EXP,
                                bias=bias_sb[:, col:col + 1], scale=EXP_SCALE,
                            )
                            if DEBUG and S == 1 and pair == 0 and hl == 0 and t == 5:
                                nc.sync.dma_start(out=dbg_pt[:, :], in_=pt)
                            pend.append((hl, t, pt))
                            while len(pend) > 3:
                                emit_pv(*pend.pop(0))
                    for item in pend:
                        emit_pv(*item)
                    for hl in range(2):
                        avf = avs[hl][:, :, :].rearrange("p a b -> p (a b)")
                        den = recp.tile([64, SP_W], F32, tag="den")
                        # reciprocal_approx_fast (custom DVE ucode) reads
                        # garbage from PSUM on HW: stage the denominator rows
                        # through SBUF first.
                        nc.vector.tensor_copy(out=den[:, :], in_=avf[64:128, :])
                        rec = recp.tile([64, SP_W], F32, tag="rec")
                        nc.vector.reciprocal_approx_fast(
                            out=rec[:, :], in_=den[:, :])
                        nc.vector.tensor_mul(
                            at[64 * hl:64 * hl + 64, :], avf[0:64, :], rec[:, :])

            def oproj(S):
                q0 = S * SP_W
                for it in range(8):
                    for half in range(2):
                        opf = sps.tile([128, 2, CH], F32, tag="sp", name="op")
                        op = opf[:, 0, :]
                        for pair in range(2):
                            nc.tensor.matmul(
                                op,
                                at_tiles[(S, pair)][:, it * 128:(it + 1) * 128],
                                wo_sb[:, pair, half * CH:(half + 1) * CH],
                                start=(pair == 0), stop=(pair == 1),
                            )
                        ob = obp.tile([128, CH], F32, tag="ob", name="ob")
                        if half == 0:
                            nc.scalar.activation(
                                out=ob[:, :], in_=op,
                                func=mybir.ActivationFunctionType.Copy)
                        else:
                            nc.vector.tensor_copy(out=ob[:, :], in_=op)
                        dma_eng = nc.scalar if half == 0 else nc.sync
                        dma_eng.dma_start(
                            out=out[q0 + it * 128:q0 + (it + 1) * 128,
                                    half * CH:(half + 1) * CH],
                            in_=ob[:, :],
                        )

            for _rep in range(reps):
                at_tiles.clear()
                proj_chunk(0)
                proj_chunk(1)
                strip_pair(0)
                proj_chunk(2)
                oproj(0)
                proj_chunk(3)
                strip_pair(1)
                oproj(1)
                if DEBUG:
                    nc.sync.dma_start(out=dbg_q[:, :, :], in_=qf8)
                    nc.sync.dma_start(out=dbg_k[:, :, :], in_=kf8)
                    nc.sync.dma_start(out=dbg_v[:, :, :, :], in_=v_sb)
                    for S in range(2):
                        for pair in range(2):
                            nc.sync.dma_start(out=dbg_at[:, S, pair, :], in_=at_tiles[(S, pair)])

    return nc


def _sigmoid(v):
    return 1.0 / (1.0 + np.exp(-v.astype(np.float64)))


def build_inputs(x, Wqkv, Wo, log_xi, pi_gate_logit, e_gate_logit):
    x = np.asarray(x, np.float32)
    Wqkv = np.asarray(Wqkv, np.float32)
    Wo = np.asarray(Wo, np.float32)
    log_xi = np.asarray(log_xi, np.float32)
    pi_gate_logit = np.asarray(pi_gate_logit, np.float32)
    e_gate_logit = np.asarray(e_gate_logit, np.float32)

    pi_g = _sigmoid(pi_gate_logit)
    c_h = (_sigmoid(e_gate_logit) / np.exp(log_xi.astype(np.float64)))

    Wq = Wqkv[0:1024].reshape(N_HEADS, D_HEAD, D_MODEL)
    Wk = Wqkv[1024:2048].reshape(N_HEADS, D_HEAD, D_MODEL)
    Wv = Wqkv[2048:3072].reshape(N_HEADS, D_HEAD, D_MODEL)

    f = np.arange(32)
    inv_freq = np.float64(math.pi) ** (1.0 - 2.0 * f / 64.0)
    pos = np.arange(SEQ, dtype=np.float64)

    f8 = ml_dtypes.float8_e4m3
    kidx = (np.arange(2)[:, None] * 64 + np.arange(64)[None, :]).T  # [p, i] -> k
    idm = np.zeros((64, 2, 128), np.float32)
    trim = np.zeros((64, 2, 128), np.float32)
    for p in range(64):
        for i in range(2):
            k = kidx[p, i]
            idm[p, i, k] = 240.0
            trim[p, i, :k] = -240.0
    idm = idm.astype(f8)
    trim = trim.astype(f8)
    bf = ml_dtypes.bfloat16

    in_maps = []
    xT64 = [np.ascontiguousarray(x[b].T).astype(np.float64) for b in range(BATCH)]
    xf8b = [np.ascontiguousarray((xT64[b] * SX).astype(f8)) for b in range(BATCH)]
    xb16b = [np.ascontiguousarray(xT64[b].astype(bf)) for b in range(BATCH)]
    for core in range(8):
        b, g = core // 4, core % 4
        hs = slice(4 * g, 4 * g + 4)
        qe = Wq[hs, 0::2, :].reshape(128, D_MODEL)
        qo = Wq[hs, 1::2, :].reshape(128, D_MODEL)
        ke = Wk[hs, 0::2, :].reshape(128, D_MODEL)
        ko = Wk[hs, 1::2, :].reshape(128, D_MODEL)
        wqk = np.concatenate([qe, qo, ke, ko], 0).T.astype(np.float64)
        wqk8 = np.ascontiguousarray((wqk * SW).astype(f8))
        wqk16 = np.ascontiguousarray((wqk * (SX * SW)).astype(bf))
        wv = np.ascontiguousarray(Wv[hs].reshape(256, D_MODEL).T.astype(bf))
        wo = np.ascontiguousarray(Wo[:, 256 * g:256 * (g + 1)].T.astype(bf))

        theta = pos[None, None, :] * inv_freq[None, :, None] * pi_g[4 * g:4 * g + 4, None, None]
        rotscale = SQ / (SX * SW)
        cost = (np.cos(theta) * rotscale).reshape(128, SEQ).astype(np.float32)
        sint = (np.sin(theta) * rotscale).reshape(128, SEQ).astype(np.float32)

        biast = np.empty((128, H_LOC * NT), np.float32)
        p = np.arange(128, dtype=np.float64)
        for hl in range(H_LOC):
            for t in range(NT):
                biast[:, hl * NT + t] = (c_h[4 * g + hl] * (128 * t + p)).astype(np.float32)

        in_maps.append({
            "xf8": xf8b[b], "xb16": xb16b[b],
            "wqk8": wqk8, "wqk16": wqk16, "wv": wv, "wo": wo,
            "cost": cost, "sint": sint, "biast": biast, "idm": idm, "trim": trim,
        })
    return in_maps


def kernel(x, Wqkv, Wo, log_xi, pi_gate_logit, e_gate_logit):
    in_maps = build_inputs(x, Wqkv, Wo, log_xi, pi_gate_logit, e_gate_logit)
    nc = build_program()
    nc.finalize()
    res = run_bass_kernel_spmd(nc, in_maps, list(range(8))).results
    out = np.zeros((BATCH, SEQ, D_MODEL), np.float32)
    for core in range(8):
        out[core // 4] += np.asarray(res[core]["out"])
    return out


# revision 24
# speedup vs baseline: 1.0506x; 1.0506x over previous
"""EulerCE attention Trainium2 kernel (v2: fp8 DoubleRow + bf16 + merged exps).

Sharding: data-parallel over batch (2) x head-parallel over 4 head-groups
(16 heads / 4 per group) = 8 cores. Core c: batch c//4, heads 4*(c%4)..+4.

Numerics (validated against a float64 oracle by host-side emulation):
  - Q/K projection in fp8-e4m3 DoubleRow (x*16, W*128 host-scaled), except
    positions 0:128 which get a bf16 "patch" projection (few-key softmax rows
    amplify q/k error; bf16 there drops max rel err from 2.3e-2 to 4.7e-3).
  - RoPE rotation on DVE+Pool in f32 out of PSUM; rotated q/k written
    directly as fp8 planes [128, 2(even/odd), n] for DoubleRow scores —
    scale 8/(16*128) folded into the host cos/sin tables.
  - Scores s^T[k,q] via fp8 DoubleRow (contraction 64 = 32 partitions x 2
    planes per head); decay bias folded into the exp bias column; exp scale
    1/512 = 1/(8*8*sqrt(64)).
  - One exp per (head, key-tile) covering the whole valid 1024-col q range
    of a strip-pair (contiguous over the 2 PSUM banks) -> bf16 P.
  - V path fully bf16 (max-normalized error is dominated by early-q rows
    where attention passes V through unaveraged; fp8 V fails the gate).
  - PV with the 64-ones-column denominator trick; reciprocal_approx_fast +
    tensor_mul -> bf16 attn^T; O-projection bf16; output DMA'd from PSUM.
"""

import sys

sys.path.insert(0, "/opt/trn_rl_repo")

import math

import ml_dtypes
import numpy as np

import concourse.bass as bass
from concourse import bacc
import concourse.mybir as mybir
import concourse.tile as tile
from concourse.bass_utils import run_bass_kernel_spmd

F32 = mybir.dt.float32
BF16 = mybir.dt.bfloat16
F8 = mybir.dt.float8e4
EXP = mybir.ActivationFunctionType.Exp
DR = mybir.MatmulPerfMode.DoubleRow

D_MODEL = 1024
N_HEADS = 16
D_HEAD = 64
BATCH = 2
SEQ = 2048
H_LOC = 4          # heads per core
CH = 512           # proj n-chunk size
KT = 128           # key tile
NT = SEQ // KT     # 16 key tiles
SP_W = 1024        # strip-pair q width
PATCH = 128        # bf16-patched positions

SX = 16.0          # fp8 scale on x
SW = 128.0         # fp8 scale on Wq/Wk
SQ = 8.0           # fp8 scale on rotated q/k
EXP_SCALE = 1.0 / (SQ * SQ * 8.0)


DEBUG = False


def build_program(reps=1):
    nc = bacc.Bacc()
    xf8 = nc.dram_tensor("xf8", [D_MODEL, SEQ], F8, kind="ExternalInput")
    xb16 = nc.dram_tensor("xb16", [D_MODEL, SEQ], BF16, kind="ExternalInput")
    wqk8 = nc.dram_tensor("wqk8", [D_MODEL, 512], F8, kind="ExternalInput")
    wqk16 = nc.dram_tensor("wqk16", [D_MODEL, 512], BF16, kind="ExternalInput")
    wv = nc.dram_tensor("wv", [D_MODEL, 256], BF16, kind="ExternalInput")
    wo = nc.dram_tensor("wo", [256, D_MODEL], BF16, kind="ExternalInput")
    cost = nc.dram_tensor("cost", [128, SEQ], BF16, kind="ExternalInput")
    sint = nc.dram_tensor("sint", [128, SEQ], BF16, kind="ExternalInput")
    biast = nc.dram_tensor("biast", [128, H_LOC * NT], F32, kind="ExternalInput")
    idm = nc.dram_tensor("idm", [64, 2, 128], F8, kind="ExternalInput")
    trim = nc.dram_tensor("trim", [64, 2, 128], F8, kind="ExternalInput")
    out = nc.dram_tensor("out", [SEQ, D_MODEL], F32, kind="ExternalOutput")
    dbg_q = nc.dram_tensor("dbg_q", [128, 2, SEQ], F8, kind="ExternalOutput") if DEBUG else None
    dbg_k = nc.dram_tensor("dbg_k", [128, 2, SEQ], F8, kind="ExternalOutput") if DEBUG else None
    dbg_v = nc.dram_tensor("dbg_v", [128, NT, H_LOC, 128], BF16, kind="ExternalOutput") if DEBUG else None
    dbg_at = nc.dram_tensor("dbg_at", [128, 2, 2, SP_W], BF16, kind="ExternalOutput") if DEBUG else None
    dbg_pt = nc.dram_tensor("dbg_pt", [128, SP_W], BF16, kind="ExternalOutput") if DEBUG else None

    with tile.TileContext(nc) as tc:
        with (
            tc.tile_pool(name="consts", bufs=1) as consts,
            tc.tile_pool(name="persist", bufs=1) as persist,
            tc.tile_pool(name="xch8", bufs=2) as xchp8,
            tc.tile_pool(name="xch16", bufs=2) as xchp16,
            tc.tile_pool(name="rot", bufs=2) as rotp,
            tc.tile_pool(name="grot", bufs=2) as grotp,
            tc.tile_pool(name="ptp", bufs=5) as ptp,
            tc.tile_pool(name="attnp", bufs=2) as attnp,
            tc.tile_pool(name="recp", bufs=2) as recp,
            tc.tile_pool(name="obp", bufs=3) as obp,
            tc.tile_pool(name="sps", bufs=2, space="PSUM") as sps,
            tc.tile_pool(name="avps", bufs=2, space="PSUM") as avps,
        ):
            # ---- constants + early x chunks, in critical-path order ----
            def load_xchunk(c):
                c0 = c * CH
                xc8 = xchp8.tile([128, 8, CH], F8, tag="xc8", name=f"xc8_{c}")
                nc.sync.dma_start(out=xc8, in_=xf8[:, c0:c0 + CH].rearrange("(k p) m -> p k m", p=128))
                xc16 = xchp16.tile([128, 8, CH], BF16, tag="xc16", name=f"xc16_{c}")
                nc.sync.dma_start(out=xc16, in_=xb16[:, c0:c0 + CH].rearrange("(k p) m -> p k m", p=128))
                return xc8, xc16

            wqk8_sb = consts.tile([128, 8, 512], F8, tag="wqk8")
            nc.sync.dma_start(out=wqk8_sb, in_=wqk8.rearrange("(k p) m -> p k m", p=128))
            xch01 = [load_xchunk(0)]
            cos_sb = consts.tile([128, SEQ], BF16, tag="cos")
            nc.sync.dma_start(out=cos_sb, in_=cost[:, :])
            sin_sb = consts.tile([128, SEQ], BF16, tag="sin")
            nc.sync.dma_start(out=sin_sb, in_=sint[:, :])
            wqk16_sb = consts.tile([128, 8, 512], BF16, tag="wqk16")
            nc.sync.dma_start(out=wqk16_sb, in_=wqk16.rearrange("(k p) m -> p k m", p=128))
            wv_sb = consts.tile([128, 8, 256], BF16, tag="wv")
            nc.sync.dma_start(out=wv_sb, in_=wv.rearrange("(k p) m -> p k m", p=128))
            xch01.append(load_xchunk(1))
            bias_sb = consts.tile([128, H_LOC * NT], F32, tag="bias")
            nc.sync.dma_start(out=bias_sb, in_=biast[:, :])
            idm_sb = consts.tile([64, 2, 128], F8, tag="idm")
            nc.sync.dma_start(out=idm_sb, in_=idm[:, :, :])
            trim_sb = consts.tile([64, 2, 128], F8, tag="trim")
            nc.sync.dma_start(out=trim_sb, in_=trim[:, :, :])
            wo_sb = consts.tile([128, 2, D_MODEL], BF16, tag="wo")
            nc.sync.dma_start(out=wo_sb, in_=wo.rearrange("(k p) m -> p k m", p=128))

            # V in [key, dh] layout; cols 64:128 ones (denominator trick)
            v_sb = persist.tile([128, NT, H_LOC, 128], BF16, tag="vsb")
            nc.vector.memset(v_sb[:, :, :, 64:128], 1.0)

            # rotated q/k, fp8 planes: [128p(=4 heads x 32), 2(even/odd), n]
            qf8 = persist.tile([128, 2, SEQ], F8, tag="qf8")
            kf8 = persist.tile([128, 2, SEQ], F8, tag="kf8")
            # bf16 patch of rotated q/k for positions 0:PATCH
            qp16 = persist.tile([128, 2, PATCH], BF16, tag="qp16")
            kp16 = persist.tile([128, 2, PATCH], BF16, tag="kp16")

            at_tiles = {}

            def rotate(pe, po, dst, c0):
                # dst fp8 planes; scale folded into cos/sin tables. DVE does
                # the PSUM-reading muls (bf16 intermediates), Pool (no PSUM
                # access) does the SBUF-only combine into fp8.
                cs = cos_sb[:, c0:c0 + CH]
                sn = sin_sb[:, c0:c0 + CH]
                t1 = rotp.tile([128, CH], BF16, tag="t1")
                t2 = rotp.tile([128, CH], BF16, tag="t2")
                t3 = grotp.tile([128, CH], BF16, tag="t3")
                t4 = grotp.tile([128, CH], BF16, tag="t4")
                nc.vector.tensor_mul(t1[:, :], pe[:, :], cs)
                nc.vector.tensor_mul(t2[:, :], po[:, :], sn)
                nc.gpsimd.tensor_sub(dst[:, 0, c0:c0 + CH], t1[:, :], t2[:, :])
                nc.vector.tensor_mul(t3[:, :], pe[:, :], sn)
                nc.vector.tensor_mul(t4[:, :], po[:, :], cs)
                nc.gpsimd.tensor_add(dst[:, 1, c0:c0 + CH], t3[:, :], t4[:, :])

            def patch_block(xc16):
                # bf16 proj + rotate for positions 0:PATCH (chunk 0 only);
                # all 4 M-blocks packed into one PSUM bank (sequential chains)
                # HW PSUM zero-regions are bank-wide: arm the bank once (the
                # very first matmul), later chains rely on zero-on-first-touch
                # of still-pending bytes, so their start stays False.
                pfull = avps.tile([128, 2, CH], F32, tag="avp", name="patch")
                pq = []
                for m in range(4):
                    p = pfull[:, 0, m * PATCH:(m + 1) * PATCH]
                    for j in range(8):
                        nc.tensor.matmul(
                            p,
                            wqk16_sb[:, j, m * 128:(m + 1) * 128],
                            xc16[:, j, 0:PATCH],
                            start=(m == 0 and j == 0), stop=(j == 7),
                            skip_group_check=True,
                        )
                    pq.append(p)
                cs = cos_sb[:, 0:PATCH]
                sn = sin_sb[:, 0:PATCH]
                for pe, po, dst in ((pq[0], pq[1], qp16), (pq[2], pq[3], kp16)):
                    t1 = rotp.tile([128, PATCH], F32, tag="t1")
                    t2 = rotp.tile([128, PATCH], F32, tag="t2")
                    nc.vector.tensor_mul(t1[:, :], pe, cs)
                    nc.vector.tensor_mul(t2[:, :], po, sn)
                    nc.vector.tensor_sub(dst[:, 0, :], t1[:, :], t2[:, :])
                    t3 = grotp.tile([128, PATCH], F32, tag="t3p")
                    t4 = grotp.tile([128, PATCH], F32, tag="t4p")
                    nc.vector.tensor_mul(t3[:, :], pe, sn)
                    nc.vector.tensor_mul(t4[:, :], po, cs)
                    nc.vector.tensor_add(dst[:, 1, :], t3[:, :], t4[:, :])

            def proj_chunk(c, pre=None):
                c0 = c * CH
                xc8, xc16 = pre if pre is not None else load_xchunk(c)
                for mp, dst in ((0, qf8), (1, kf8)):  # QE/QO then KE/KO, fp8 DR
                    pqk = sps.tile([128, 2, CH], F32, tag="sp", name=f"qk{mp}")
                    for half in range(2):
                        m = 2 * mp + half
                        for j in range(4):
                            nc.tensor.matmul(
                                pqk[:, half, :],
                                wqk8_sb[:, 2 * j:2 * j + 2, m * 128:(m + 1) * 128],
                                xc8[:, 2 * j:2 * j + 2, :],
                                start=(j == 0), stop=(j == 3),
                                perf_mode=DR,
                            )
                    rotate(pqk[:, 0, :], pqk[:, 1, :], dst, c0)
                if c == 0:
                    patch_block(xc16)
                for kt in range(4):  # V projection, bf16
                    t = 4 * c + kt
                    vpf = avps.tile([128, 2, CH], F32, tag="avp", name="vp")
                    vp = vpf[:, 0, 0:256]
                    for j in range(8):
                        nc.tensor.matmul(
                            vp,
                            xc16[:, j, kt * 128:(kt + 1) * 128],
                            wv_sb[:, j, :],
                            start=(j == 0), stop=(j == 7),
                        )
                    nc.vector.tensor_copy(
                        out=v_sb[:, t, :, 0:64],
                        in_=vp.rearrange("p (h d) -> p h d", h=4),
                    )

            def strip_pair(S):
                q0 = S * SP_W
                T = 8 * S + 8  # key tiles in range
                for pair in range(2):
                    at = attnp.tile([128, SP_W], BF16, tag=f"at{pair}",
                                    name=f"at_{S}_{pair}")
                    at_tiles[(S, pair)] = at
                    avs = [
                        avps.tile([128, 2, CH], F32, tag="avp",
                                  name=f"av_{S}_{pair}_{hl}")
                        for hl in range(2)
                    ]
                    def emit_pv(fhl, ft, fpt):
                        fh = 2 * pair + fhl
                        for half in range(2):
                            hq0 = q0 + half * CH
                            qoff = max(0, KT * ft - hq0)
                            if qoff >= CH:
                                continue
                            t_hi = 8 * S + 4 * (half + 1)
                            nc.tensor.matmul(
                                avs[fhl][:, half, qoff:CH],
                                v_sb[:, ft, fh, :],
                                fpt[:, half * CH + qoff:(half + 1) * CH],
                                start=(ft == 0), stop=(ft == t_hi - 1),
                            )

                    pend = []  # deferred PV work: (hl, t, pt)
                    for t in range(T):
                        for hl in range(2):
                            h = 2 * pair + hl
                            r0 = 32 * h
                            sp = sps.tile([128, 2, CH], F32, tag="sp",
                                          name=f"sp{hl}")
                            o = max(0, KT * t - q0)  # first valid col
                            for half in range(2):
                                hq0 = q0 + half * CH
                                qoff = max(0, KT * t - hq0)
                                if qoff >= CH:
                                    continue
                                if S == 0 and t == 0 and half == 0:
                                    # bf16 patch for q 0:128 then fp8 rest
                                    nc.tensor.matmul(
                                        sp[:, 0, 0:PATCH],
                                        kp16[r0:r0 + 32, 0, :],
                                        qp16[r0:r0 + 32, 0, :],
                                        start=True, stop=False,
                                        tile_position=(r0, 0),
                                    )
                                    nc.tensor.matmul(
                                        sp[:, 0, 0:PATCH],
                                        kp16[r0:r0 + 32, 1, :],
                                        qp16[r0:r0 + 32, 1, :],
                                        start=False, stop=True,
                                        tile_position=(r0, 0),
                                    )
                                    # same bank as the patch chain above:
                                    # start=False, bytes are still pending
                                    # from its bank-wide arm (zero-on-touch)
                                    nc.tensor.matmul(
                                        sp[:, 0, PATCH:CH],
                                        kf8[r0:r0 + 32, :, 0:KT],
                                        qf8[r0:r0 + 32, :, PATCH:CH],
                                        start=False, stop=True,
                                        perf_mode=DR,
                                        tile_position=(r0, 0),
                                        skip_group_check=True,
                                    )
                                    nc.tensor.matmul(
                                        sp[:, 0, 0:KT],
                                        idm_sb[:, :, :], trim_sb[:, :, :],
                                        start=False, stop=True,
                                        perf_mode=DR,
                                        skip_group_check=True,
                                    )
                                    continue
                                nc.tensor.matmul(
                                    sp[:, half, qoff:CH],
                                    kf8[r0:r0 + 32, :, t * KT:(t + 1) * KT],
                                    qf8[r0:r0 + 32, :, hq0 + qoff:hq0 + CH],
                                    start=True, stop=True,
                                    perf_mode=DR,
                                    tile_position=(r0, 0),
                                )
                                if KT * t >= q0 and (o // CH) == half:
                                    # causal mask folded into the score PSUM:
                                    # identity(+240) x lower-tri(-240) adds
                                    # -57600 on masked (q < k) elements; exp
                                    # then yields exactly 0 there.
                                    om = o % CH
                                    nc.tensor.matmul(
                                        sp[:, half, om:om + KT],
                                        idm_sb[:, :, :], trim_sb[:, :, :],
                                        start=False, stop=True,
                                        perf_mode=DR,
                                        skip_group_check=True,
                                    )
                            pt = ptp.tile([128, SP_W], BF16, tag="pt",
                                          name=f"pt{hl}")
                            col = h * NT + t
                            spf = sp[:, :, :].rearrange("p a b -> p (a b)")
                            nc.scalar.activation(
                                out=pt[:, o:SP_W], in_=spf[:, o:SP_W],
                                func=EXP,
                                bias=bias_sb[:, col:col + 1], scale=EXP_SCALE,
                            )
                            if DEBUG and S == 1 and pair == 0 and hl == 0 and t == 5:
                                nc.sync.dma_start(out=dbg_pt[:, :], in_=pt)
                            pend.append((hl, t, pt))
                            while len(pend) > 3:
                                emit_pv(*pend.pop(0))
                    for item in pend:
                        emit_pv(*item)
                    for hl in range(2):
                        avf = avs[hl][:, :, :].rearrange("p a b -> p (a b)")
                        den = recp.tile([64, SP_W], F32, tag="den")
                        # reciprocal_approx_fast (custom DVE ucode) reads
                        # garbage from PSUM on HW: stage the denominator rows
                        # through SBUF first.
                        nc.vector.tensor_copy(out=den[:, :], in_=avf[64:128, :])
                        rec = recp.tile([64, SP_W], F32, tag="rec")
                        nc.vector.reciprocal_approx_fast(
                            out=rec[:, :], in_=den[:, :])
                        nc.vector.tensor_mul(
                            at[64 * hl:64 * hl + 64, :], avf[0:64, :], rec[:, :])

            def oproj(S):
                q0 = S * SP_W
                for it in range(8):
                    for half in range(2):
                        opf = sps.tile([128, 2, CH], F32, tag="sp", name="op")
                        op = opf[:, 0, :]
                        for pair in range(2):
                            nc.tensor.matmul(
                                op,
                                at_tiles[(S, pair)][:, it * 128:(it + 1) * 128],
                                wo_sb[:, pair, half * CH:(half + 1) * CH],
                                start=(pair == 0), stop=(pair == 1),
                            )
                        ob = obp.tile([128, CH], F32, tag="ob", name="ob")
                        if half == 0:
                            nc.scalar.activation(
                                out=ob[:, :], in_=op,
                                func=mybir.ActivationFunctionType.Copy)
                        else:
                            nc.vector.tensor_copy(out=ob[:, :], in_=op)
                        nc.sync.dma_start(
                            out=out[q0 + it * 128:q0 + (it + 1) * 128,
                                    half * CH:(half + 1) * CH],
                            in_=ob[:, :],
                        )

            for _rep in range(reps):
                at_tiles.clear()
                proj_chunk(0, xch01[0])
                proj_chunk(1, xch01[1])
                strip_pair(0)
                proj_chunk(2)
                oproj(0)
                proj_chunk(3)
                strip_pair(1)
                oproj(1)
                if DEBUG:
                    nc.sync.dma_start(out=dbg_q[:, :, :], in_=qf8)
                    nc.sync.dma_start(out=dbg_k[:, :, :], in_=kf8)
                    nc.sync.dma_start(out=dbg_v[:, :, :, :], in_=v_sb)
                    for S in range(2):
                        for pair in range(2):
                            nc.sync.dma_start(out=dbg_at[:, S, pair, :], in_=at_tiles[(S, pair)])

    return nc


def _sigmoid(v):
    return 1.0 / (1.0 + np.exp(-v.astype(np.float64)))


def build_inputs(x, Wqkv, Wo, log_xi, pi_gate_logit, e_gate_logit):
    x = np.asarray(x, np.float32)
    Wqkv = np.asarray(Wqkv, np.float32)
    Wo = np.asarray(Wo, np.float32)
    log_xi = np.asarray(log_xi, np.float32)
    pi_gate_logit = np.asarray(pi_gate_logit, np.float32)
    e_gate_logit = np.asarray(e_gate_logit, np.float32)

    pi_g = _sigmoid(pi_gate_logit)
    c_h = (_sigmoid(e_gate_logit) / np.exp(log_xi.astype(np.float64)))

    Wq = Wqkv[0:1024].reshape(N_HEADS, D_HEAD, D_MODEL)
    Wk = Wqkv[1024:2048].reshape(N_HEADS, D_HEAD, D_MODEL)
    Wv = Wqkv[2048:3072].reshape(N_HEADS, D_HEAD, D_MODEL)

    f = np.arange(32)
    inv_freq = np.float64(math.pi) ** (1.0 - 2.0 * f / 64.0)
    pos = np.arange(SEQ, dtype=np.float64)

    f8 = ml_dtypes.float8_e4m3
    kidx = (np.arange(2)[:, None] * 64 + np.arange(64)[None, :]).T  # [p, i] -> k
    idm = np.zeros((64, 2, 128), np.float32)
    trim = np.zeros((64, 2, 128), np.float32)
    for p in range(64):
        for i in range(2):
            k = kidx[p, i]
            idm[p, i, k] = 240.0
            trim[p, i, :k] = -240.0
    idm = idm.astype(f8)
    trim = trim.astype(f8)
    bf = ml_dtypes.bfloat16

    in_maps = []
    xT64 = [np.ascontiguousarray(x[b].T).astype(np.float64) for b in range(BATCH)]
    xf8b = [np.ascontiguousarray((xT64[b] * SX).astype(f8)) for b in range(BATCH)]
    xb16b = [np.ascontiguousarray(xT64[b].astype(bf)) for b in range(BATCH)]
    for core in range(8):
        b, g = core // 4, core % 4
        hs = slice(4 * g, 4 * g + 4)
        qe = Wq[hs, 0::2, :].reshape(128, D_MODEL)
        qo = Wq[hs, 1::2, :].reshape(128, D_MODEL)
        ke = Wk[hs, 0::2, :].reshape(128, D_MODEL)
        ko = Wk[hs, 1::2, :].reshape(128, D_MODEL)
        wqk = np.concatenate([qe, qo, ke, ko], 0).T.astype(np.float64)
        wqk8 = np.ascontiguousarray((wqk * SW).astype(f8))
        wqk16 = np.ascontiguousarray((wqk * (SX * SW)).astype(bf))
        wv = np.ascontiguousarray(Wv[hs].reshape(256, D_MODEL).T.astype(bf))
        wo = np.ascontiguousarray(Wo[:, 256 * g:256 * (g + 1)].T.astype(bf))

        theta = pos[None, None, :] * inv_freq[None, :, None] * pi_g[4 * g:4 * g + 4, None, None]
        rotscale = SQ / (SX * SW)
        cost = (np.cos(theta) * rotscale).reshape(128, SEQ).astype(ml_dtypes.bfloat16)
        sint = (np.sin(theta) * rotscale).reshape(128, SEQ).astype(ml_dtypes.bfloat16)

        biast = np.empty((128, H_LOC * NT), np.float32)
        p = np.arange(128, dtype=np.float64)
        for hl in range(H_LOC):
            for t in range(NT):
                biast[:, hl * NT + t] = (c_h[4 * g + hl] * (128 * t + p)).astype(np.float32)

        in_maps.append({
            "xf8": xf8b[b], "xb16": xb16b[b],
            "wqk8": wqk8, "wqk16": wqk16, "wv": wv, "wo": wo,
            "cost": cost, "sint": sint, "biast": biast, "idm": idm, "trim": trim,
        })
    return in_maps


def kernel(x, Wqkv, Wo, log_xi, pi_gate_logit, e_gate_logit):
    in_maps = build_inputs(x, Wqkv, Wo, log_xi, pi_gate_logit, e_gate_logit)
    nc = build_program()
    nc.finalize()
    res = run_bass_kernel_spmd(nc, in_maps, list(range(8))).results
    out = np.zeros((BATCH, SEQ, D_MODEL), np.float32)
    for core in range(8):
        out[core // 4] += np.asarray(res[core]["out"])
    return out
